# revision 10
# baseline (speedup 1.0000x reference)
"""Gaussian DPMM E-step kernel for 8 trn2 NeuronCores.

Data-parallel over N: each core handles 32768 rows of x.
Host precomputes all K/D-sized variational quantities in float64
(digamma/gammaln etc.), folds them into two [D,K] matmul weights, a
[K] constant row (shipped as a 3-way bf16 split so the PE can add it
via a C=3 matmul), and the scalar KL terms. The device computes, per
128-row tile:  y = x^2 @ Wa + x @ Wb + Ck   (PSUM accumulation),
row-softmax over K with a fused exp+row-sum on the scalar engine, and
ships back r plus per-row (-max, sum) so the host can assemble the
ELBO log-norm term in float64.
"""

import numpy as np
import ml_dtypes
from scipy.special import digamma, gammaln

import concourse.bass as bass
import concourse.bacc as bacc
import concourse.mybir as mybir
from concourse.tile import TileContext
from concourse.bass_utils import run_bass_kernel_spmd

N, K, D = 262144, 128, 64
NCORES = 8
NS = N // NCORES          # 32768 rows per core
ROWS_PER_GROUP = 512
G = NS // ROWS_PER_GROUP  # 64 groups per core
JT = ROWS_PER_GROUP // 128  # 4 tiles of 128 rows per group

ALPHA_DP = 1.0
TAU0 = 0.0
C0 = 1.0
N0 = float(D + 2)
B0 = 1.0
LOG2PI = float(np.log(2.0 * np.pi))

F32 = mybir.dt.float32
BF16 = mybir.dt.bfloat16

_NC_CACHE = None
LAST_RESULT = None  # BassKernelResults of the most recent run (for test.py)


def _build_bass():
    nc = bacc.Bacc(None, target_bir_lowering=False)

    XT = nc.dram_tensor("xt", [G, D, ROWS_PER_GROUP], F32, kind="ExternalInput")
    WA = nc.dram_tensor("wa", [D, K], F32, kind="ExternalInput")
    WB = nc.dram_tensor("wb", [D, K], F32, kind="ExternalInput")
    # [3, 2K]: columns 0..K-1 are ones, columns K..2K-1 are the 3-way
    # bf16 split of Ck — one DMA so the const matmul has a single dep.
    CKS = nc.dram_tensor("cks", [3, 2 * K], BF16, kind="ExternalInput")
    R = nc.dram_tensor("r", [NS, K], F32, kind="ExternalOutput")
    NMX = nc.dram_tensor("nmx", [G, 128, JT], F32, kind="ExternalOutput")
    SSUM = nc.dram_tensor("ssum", [G, 128, JT], F32, kind="ExternalOutput")

    with TileContext(nc) as tc:
        with (
            tc.tile_pool(name="const", bufs=1) as cpool,
            tc.tile_pool(name="xin", bufs=4) as xpool,
            tc.tile_pool(name="xsq", bufs=4) as x2pool,
            tc.tile_pool(name="ebuf", bufs=4) as epool,
            tc.tile_pool(name="rbuf", bufs=4) as rpool,
            tc.tile_pool(name="small", bufs=8) as spool,
            tc.tile_pool(name="ps", bufs=4, space="PSUM") as pspool,
        ):
            wa = cpool.tile([D, K], F32)
            nc.sync.dma_start(wa[:], WA[:])
            wb = cpool.tile([D, K], F32)
            nc.sync.dma_start(wb[:], WB[:])
            onck = cpool.tile([3, 2 * K], BF16)
            nc.sync.dma_start(onck[:], CKS[:])
            ones3 = onck[:, 0:K]
            cks = onck[:, K:2 * K]

            # Warmup matmuls: let the PE observe the weight DMAs here so
            # loop matmuls carry at most one sync wait each (the fused
            # fp32 LDWEIGHTS slot only accepts a single wait command).
            warm = pspool.tile([128, K], F32, tag="warm")
            nc.tensor.matmul(warm[:], wa[:], wa[:], start=True, stop=False)
            nc.tensor.matmul(warm[:], wb[:], wb[:], start=False, stop=False)
            nc.tensor.matmul(warm[:], ones3, cks, start=False, stop=True)

            for g in range(G):
                xt = xpool.tile([D, ROWS_PER_GROUP], F32, tag="xt")
                nc.sync.dma_start(xt[:], XT[g, :, :])
                x2 = x2pool.tile([D, ROWS_PER_GROUP], F32, tag="x2")
                nc.gpsimd.tensor_mul(x2[:], xt[:], xt[:])

                ps = pspool.tile([128, JT, K], F32, tag="ps")
                for j in range(JT):
                    sl = slice(j * 128, (j + 1) * 128)
                    nc.tensor.matmul(ps[:, j, :], ones3, cks,
                                     start=True, stop=False)
                    nc.tensor.matmul(ps[:, j, :], x2[:, sl], wa[:],
                                     start=False, stop=False)
                    nc.tensor.matmul(ps[:, j, :], xt[:, sl], wb[:],
                                     start=False, stop=True)

                negmx = spool.tile([128, JT], F32, tag="negmx")
                nc.vector.reduce_max(negmx[:], ps[:],
                                     axis=mybir.AxisListType.X, negate=True)

                e = epool.tile([128, JT, K], F32, tag="e")
                s = spool.tile([128, JT], F32, tag="s")
                for j in range(JT):
                    nc.scalar.activation(
                        e[:, j, :], ps[:, j, :],
                        mybir.ActivationFunctionType.Exp,
                        bias=negmx[:, j:j + 1], scale=1.0,
                        accum_out=s[:, j:j + 1])

                sinv = spool.tile([128, JT], F32, tag="sinv")
                nc.vector.reciprocal(sinv[:], s[:])

                # r = e * sinv  (broadcast sinv over the K axis)
                r = rpool.tile([128, JT, K], F32, tag="r")
                si = sinv[:]
                si_b = bass.AP(si.tensor, si.offset, list(si.ap) + [[0, K]])
                nc.vector.tensor_mul(r[:], e[:], si_b)

                nc.sync.dma_start(
                    R[g * ROWS_PER_GROUP:(g + 1) * ROWS_PER_GROUP, :]
                    .rearrange("(j p) k -> p j k", p=128),
                    r[:])
                nc.sync.dma_start(NMX[g, :, :], negmx[:])
                nc.sync.dma_start(SSUM[g, :, :], s[:])

    nc.finalize()  # run bacc passes (reg alloc, multi-wait legalization)
    return nc


def _get_nc():
    global _NC_CACHE
    if _NC_CACHE is None:
        _NC_CACHE = _build_bass()
    return _NC_CACHE


def _host_params(nat_u, nat_v, nat_tau, nat_c, nat_n, nat_B):
    """Replicate the reference's natural->common + per-cluster terms in f64."""
    u = nat_u.astype(np.float64) + 1.0
    v = nat_v.astype(np.float64) + 1.0
    c = nat_c.astype(np.float64)
    tau = nat_tau.astype(np.float64) / c[:, None]
    n = nat_n.astype(np.float64) - D - 2.0
    B = nat_B.astype(np.float64) - c[:, None] * tau ** 2

    dg_uv = digamma(u + v)
    e_log_stick = digamma(u) - dg_uv
    e_log_1m = digamma(v) - dg_uv
    e_log_pi = e_log_stick + np.cumsum(e_log_1m) - e_log_1m

    e_log_det = D * digamma(0.5 * n) - np.sum(np.log(0.5 * B), axis=1)
    nb = n[:, None] / B                      # [K, D]

    Wa = np.ascontiguousarray((-0.5 * nb).T.astype(np.float32))       # [D, K]
    Wb = np.ascontiguousarray((tau * nb).T.astype(np.float32))        # [D, K]
    Ck = (e_log_pi + 0.5 * (e_log_det - D * LOG2PI)
          - 0.5 * (np.sum(tau ** 2 * nb, axis=1) + D / c))            # [K]

    # 3-way bf16 split of Ck so a C=3 bf16 matmul reconstructs it to
    # ~2^-24 relative accuracy inside the PSUM accumulation.
    ch = Ck.astype(ml_dtypes.bfloat16)
    cm = (Ck - ch.astype(np.float64)).astype(ml_dtypes.bfloat16)
    cl = (Ck - ch.astype(np.float64) - cm.astype(np.float64)).astype(
        ml_dtypes.bfloat16)
    cksplit = np.stack([ch, cm, cl], axis=0)                          # [3, K]
    ones3 = np.ones((3, K), dtype=ml_dtypes.bfloat16)
    CkS = np.ascontiguousarray(np.concatenate([ones3, cksplit], axis=1))

    # KL terms (scalars), float64.
    a0, b0 = 1.0, ALPHA_DP
    kl_beta = np.sum(
        gammaln(u + v) - gammaln(u) - gammaln(v)
        - (gammaln(a0 + b0) - gammaln(a0) - gammaln(b0))
        + (u - a0) * digamma(u) + (v - b0) * digamma(v)
        + (a0 + b0 - u - v) * dg_uv)

    a1 = 0.5 * n[:, None]
    b1 = 0.5 * B
    a0g = 0.5 * N0
    b0g = 0.5 * B0
    kl_gamma = np.sum(
        (a1 - a0g) * digamma(a1) - gammaln(a1) + gammaln(a0g)
        + a0g * (np.log(b1) - np.log(b0g)) + a1 * (b0g - b1) / b1)

    kl_norm = 0.5 * np.sum(
        np.log(c[:, None] / C0) + C0 / c[:, None] - 1.0
        + C0 * nb * (tau - TAU0) ** 2)

    kl_total = kl_beta + kl_gamma + kl_norm
    return Wa, Wb, CkS, kl_total


def time_device_only(inputs_np, iters=5):
    """Best-effort device execution time estimate in ns.

    Re-runs the cached PJRT executable with fixed in_maps and times the
    run_bass_kernel_spmd call. Includes axon dispatch + host<->device
    transfer, so it is an upper bound on NEFF exec time.
    """
    import time as _time
    x = np.asarray(inputs_np["x"], dtype=np.float32)
    Wa, Wb, CkS, _ = _host_params(
        np.asarray(inputs_np["nat_u"]), np.asarray(inputs_np["nat_v"]),
        np.asarray(inputs_np["nat_tau"]), np.asarray(inputs_np["nat_c"]),
        np.asarray(inputs_np["nat_n"]), np.asarray(inputs_np["nat_B"]))
    XT_all = np.ascontiguousarray(
        x.reshape(NCORES, G, ROWS_PER_GROUP, D).transpose(0, 1, 3, 2))
    in_maps = [{"xt": XT_all[i], "wa": Wa, "wb": Wb, "cks": CkS}
               for i in range(NCORES)]
    nc = _get_nc()
    best = float("inf")
    for _ in range(iters):
        t0 = _time.time()
        run_bass_kernel_spmd(nc, in_maps, core_ids=list(range(NCORES)))
        best = min(best, _time.time() - t0)
    return best * 1e9


def kernel(x, nat_u, nat_v, nat_tau, nat_c, nat_n, nat_B):
    global LAST_RESULT
    x = np.asarray(x, dtype=np.float32)
    Wa, Wb, CkS, kl_total = _host_params(
        np.asarray(nat_u), np.asarray(nat_v), np.asarray(nat_tau),
        np.asarray(nat_c), np.asarray(nat_n), np.asarray(nat_B))

    # Pre-transpose x per core: [NCORES, G, D, 512] contiguous.
    XT_all = np.ascontiguousarray(
        x.reshape(NCORES, G, ROWS_PER_GROUP, D).transpose(0, 1, 3, 2))

    in_maps = [
        {"xt": XT_all[i], "wa": Wa, "wb": Wb, "cks": CkS}
        for i in range(NCORES)
    ]
    nc = _get_nc()
    res = run_bass_kernel_spmd(nc, in_maps, core_ids=list(range(NCORES)))
    LAST_RESULT = res

    r_full = np.concatenate([res.results[i]["r"] for i in range(NCORES)], axis=0)

    ll = 0.0
    for i in range(NCORES):
        nmx = res.results[i]["nmx"].astype(np.float64)
        ss = res.results[i]["ssum"].astype(np.float64)
        ll += np.sum(-nmx + np.log(ss))

    elbo = ll - kl_total
    return r_full, np.array(-elbo, dtype=np.float32)
